# revision 23
# baseline (speedup 1.0000x reference)
"""GNN message-passing (R-GCN style) kernel for 8 Trainium2 NeuronCores.

Reference computation:
    msgs = einsum("eoi,ei->eo", W[widx], x[u])      # per-edge transform
    out  = relu(segment_sum(msgs, v, N))            # scatter-add + relu

Distribution strategy: edges are sharded by destination-node range
(12500 nodes per core), so each core owns a disjoint slice of the output
and no inter-core collective is needed.  W and x are replicated.

Device-side work (all FLOPs):
  Launch A: per-edge weight transform.  Weight groups are packed four to
    a matmul ("quads", paired by size so padding stays small): the
    [128,128] block-diagonal lhsT holds each quad member's 16x16 weight
    on two of the eight j-slots, and each rhs column carries 8 edges
    (2 per member group).  This quarters the stationary-weight traffic
    through the PE (the dominant tensor cost) versus one group per
    matmul, and shrinks the SBUF operand to 2.1MB so the zero-fill is
    cheap.  The operand layout bd[16j+i, 1024j + 64o + Q] makes every
    lhsT a single-stride access pattern AND every stripe build a fully
    contiguous [16, 1024] DMA from the 0.26MB host bank W8.  Quad column
    ranges are sized to the actual per-group edge counts (maxed across
    cores so one SPMD program serves all 8 cores).  Each quad owns one
    [128, 512] PSUM tile drained right after its matmul pieces by copies
    alternating between the vector and scalar engines.
  Launch B: segment-sum + ReLU.  Destination nodes are bucketed into
    128-node windows by descending degree, so each window is padded only
    to its own max degree DN_k (rounded to a multiple of 4); two
    pairwise bf16 add levels (DVE 2x mode) halve the slots twice, then a
    short X-reduce finishes each window batch.  The input is stored
    partition-major so every window-run load is one contiguous 2D DMA.
    ReLU on the scalar engine, one contiguous output store at the end.

The host does data layout only: sharding, sorting/padding into the
static structures, gathering x rows into the packed matmul operand, and
permuting the 16-float messages from widx-order to v-order between the
two launches.  (Device-side per-edge random access is not available:
the loadable GPSIMD ucode libraries are absent and indirect DMA has
32B/row descriptor granularity, far too slow for 200K rows/core.)
"""

import sys

sys.path.insert(0, "/opt/trn_rl_repo")

import numpy as np
import ml_dtypes

try:
    # bass_utils imports antenv.axon_hooks when tracing is requested via
    # env; some images lack that module — register a graceful stub so a
    # BASS_TRACE=1 environment degrades to "no trace" instead of crashing.
    import antenv.axon_hooks  # noqa: F401
except ImportError:
    import types

    import antenv

    _hooks = types.ModuleType("antenv.axon_hooks")
    _hooks._hook = None
    _hooks.set_axon_ntff_profile_hook = lambda h: setattr(_hooks, "_hook", h)
    _hooks.get_axon_ntff_profile_hook = lambda: _hooks._hook
    sys.modules["antenv.axon_hooks"] = _hooks
    antenv.axon_hooks = _hooks

import concourse.bacc as bacc
import concourse.mybir as mybir
import concourse.tile as tile
from concourse.bass_utils import run_bass_kernel_spmd

BF16 = ml_dtypes.bfloat16

# set by test harnesses: when True, launches run with trace=True and
# per-launch exec times land in LAST_EXEC_NS
TRACE = False
LAST_EXEC_NS = []

N_NODES = 100000
D = 16
NW = 256
NQ = NW // 4                      # weight quads (4 groups per matmul)
N_CORES = 8
VSH = N_NODES // N_CORES          # 12500 destination nodes per core

CHUNK = 4096                      # A-side columns per DMA chunk (last ragged)
PSB = 512                         # A-side columns per PSUM tile
NWIN = (VSH + 127) // 128         # 98 destination 128-node windows per core
B_MAX_FREE = 6144                 # B-side max elems/partition per sbuf tile
B_OUT_BF16 = True                 # B-side reduce/relu/output in bf16 (DVE 2x)


def _build_kernel_a(TCP, pieces):
    """pieces = list of (Q, reg, c0, c1) ascending in c0: matmul piece of
    quad Q covering absolute cols [c0, c1), accumulated in PSUM tile
    (Q, reg) and drained immediately after."""
    nc = bacc.Bacc(None, target_bir_lowering=False, debug=False)
    XU = nc.dram_tensor("XU", [128, TCP], mybir.dt.bfloat16, kind="ExternalInput")
    W8 = nc.dram_tensor("W8", [128, D * NQ], mybir.dt.bfloat16, kind="ExternalInput")
    MSG = nc.dram_tensor("MSG", [128, TCP], mybir.dt.bfloat16, kind="ExternalOutput")

    by_chunk = {}
    for pc in pieces:
        by_chunk.setdefault(pc[2] // CHUNK, []).append(pc)
    nchunks = (TCP + CHUNK - 1) // CHUNK      # last chunk may be ragged

    with tile.TileContext(nc) as tc:
        with (
            tc.tile_pool(name="bd", bufs=1) as bdp,
            tc.tile_pool(name="xu", bufs=1) as xu_pool,
            tc.tile_pool(name="sbuf", bufs=1) as pool,
            tc.tile_pool(name="psum", bufs=2, space="PSUM") as psum_pool,
        ):
            # ---- striped quad weight operand in SBUF -------------------
            # bd[16j+i, 1024j + 64o + Q] = W[quad Q member j//2][o, i];
            # stripe j is the contiguous block [16j:16j+16, 1024j:1024j+1024]
            bd = bdp.tile([128, 8 * D * NQ], mybir.dt.bfloat16, tag="bd")
            # 3-way memset split so the zero-fill clears in ~1us per engine
            bdu = bd[:].bitcast(mybir.dt.uint32)      # [128, 4096]
            nc.vector.memset(bdu[:, 0:1376], 0)
            nc.gpsimd.memset(bdu[:, 1376:2752], 0)
            nc.scalar.mul(bdu[:, 2752:4096], bdu[:, 2752:4096], 0.0)
            bdv = bd[:].rearrange("p (m q) -> p m q", q=NQ)

            # all XU chunk loads issued up front on the sync queue so the
            # DMA rings stream back-to-back from the start of the launch
            xu_tiles = []
            for ch in range(nchunks):
                w = min(CHUNK, TCP - ch * CHUNK)
                xu_t = xu_pool.tile([128, w], mybir.dt.bfloat16, tag=f"xu{ch}")
                nc.sync.dma_start(out=xu_t[:], in_=XU[:, ch * CHUNK:ch * CHUNK + w])
                xu_tiles.append(xu_t)
            # stripe builds spread over the scalar/gpsimd queues (their
            # sequencers are otherwise idle; sync is busy with XU)
            stripe_q = [nc.scalar, nc.gpsimd]
            for j in range(8):
                stripe_q[j % 2].dma_start(
                    out=bd[16 * j:16 * (j + 1), 1024 * j:1024 * (j + 1)],
                    in_=W8[16 * j:16 * (j + 1), :],
                )
            ncopy = 0
            nstore = 0
            ps_tiles = {}
            for ch in range(nchunks):
                base = ch * CHUNK
                w = min(CHUNK, TCP - base)
                xu_t = xu_tiles[ch]
                out_t = pool.tile([128, w], mybir.dt.bfloat16, tag=f"out{ch}")
                for Q, reg, c0, c1 in by_chunk.get(ch, []):
                    key = (Q, reg)
                    if key not in ps_tiles:
                        ps_new = psum_pool.tile([128, PSB], mybir.dt.float32,
                                                tag=f"ps{len(ps_tiles) % 4}")
                        ps_tiles[key] = (ps_new, c0)  # c0 = tile col origin
                    ps, orig = ps_tiles[key]
                    nc.tensor.matmul(
                        out=ps[:, c0 - orig:c1 - orig],
                        lhsT=bdv[:, :, Q:Q + 1],
                        rhs=xu_t[:, c0 - base:c1 - base],
                        start=True,
                        stop=True,
                    )
                    if ncopy % 2 == 0:
                        nc.vector.tensor_copy(
                            out_t[:, c0 - base:c1 - base], ps[:, c0 - orig:c1 - orig])
                    else:
                        nc.scalar.copy(
                            out=out_t[:, c0 - base:c1 - base],
                            in_=ps[:, c0 - orig:c1 - orig])
                    ncopy += 1
                # half-chunk stores alternating between the gpsimd (SWDGE)
                # and sync (HWDGE) queues
                for h0, h1 in ((0, w // 2), (w // 2, w)):
                    q = nc.gpsimd if nstore % 2 == 0 else nc.sync
                    q.dma_start(out=MSG[:, base + h0:base + h1],
                                in_=out_t[:, h0:h1])
                    nstore += 1
    nc.compile()
    return nc


def _build_kernel_b(PT, runs):
    """runs = list of (dn, k0, k1, poff) equal-DN window runs (chunked).
    MSGB is partition-major: MSGB[p, poff_k + o*dn_k + s] = slot s of
    component o of the node at rank k*128+p, so every window-run DMA is a
    fully contiguous 2D slice."""
    nc = bacc.Bacc(None, target_bir_lowering=False, debug=False)
    odt = mybir.dt.bfloat16 if B_OUT_BF16 else mybir.dt.float32
    MSGB = nc.dram_tensor("MSGB", [128, PT], mybir.dt.bfloat16, kind="ExternalInput")
    # partition-major output: OUTP[p, k*D+o] = out of node at rank k*128+p
    OUTP = nc.dram_tensor("OUTP", [128, NWIN * D], odt, kind="ExternalOutput")

    nruns = len(runs)
    # output split in thirds (separate tiles) so each third's store can
    # trigger as soon as the relus covering it complete
    thirds = [0, (NWIN // 3), (2 * NWIN) // 3, NWIN]

    with nc.allow_low_precision("node sums of <=44 bf16 terms fit error budget"), \
            tile.TileContext(nc) as tc:
        with (
            tc.tile_pool(name="out", bufs=1) as outp,
            tc.tile_pool(name="msg", bufs=1) as msgp,
            tc.tile_pool(name="sbuf", bufs=1) as pool,
        ):
            out_ts = []
            for i in range(3):
                out_t = outp.tile([128, (thirds[i + 1] - thirds[i]) * D], odt,
                                  tag=f"out{i}")
                out_ts.append(out_t)

            # loads issued in a size pyramid (small, ..., big, ..., small)
            # on the sync queue (HWDGE): the first tile lands early so the
            # DVE starts ~9us, cumulative loads stay ahead of the adds,
            # and the last tile is small so the compute tail is short
            asc = sorted(range(nruns),
                         key=lambda i: (runs[i][2] - runs[i][1]) * runs[i][0])
            order = asc[0::2] + asc[1::2][::-1]
            msg_ts = {}
            for ri in order:
                dn, k0, k1, poff = runs[ri]
                nw = k1 - k0
                msg_t = msgp.tile([128, nw * D * dn], mybir.dt.bfloat16,
                                  tag=f"msg{ri}")
                nc.sync.dma_start(out=msg_t[:], in_=MSGB[:, poff:poff + nw * D * dn])
                msg_ts[ri] = msg_t

            # DVE p-state warmup: a few tiny adds so the engine clock has
            # ramped before the first real level-1 lands (a cold first
            # instruction was observed running ~5x slow)
            wrm = pool.tile([128, 64], mybir.dt.bfloat16, tag="wrm")
            nc.vector.memset(wrm[:], 0)
            for _ in range(6):
                nc.vector.tensor_tensor(out=wrm[:, :32], in0=wrm[:, :32],
                                        in1=wrm[:, 32:], op=mybir.AluOpType.add)

            # add tree per run, o innermost ([p, w, s, o]): every split
            # keeps a contiguous 16-wide o run, so ALL levels (including
            # odd-remainder merges) hit the DVE 2x mode.  Odd slot counts
            # park the unpaired plane on a residual list merged at the
            # end, also at 2x.
            # level 1 of the two largest runs goes to the gpsimd engine
            # (mid-pyramid, parallel to the DVE stream); their remaining
            # DVE levels are DEFERRED two runs so the in-order DVE queue
            # never head-of-line blocks on the slow gpsimd add
            gp_l1 = set(asc[-2:])

            def dve_chain(ri, nw, cur, r, resid, lvl):
                while r > 1:
                    h = r // 2
                    if r % 2 == 1:
                        resid.append((cur, r, r - 1))
                    v = cur[:].rearrange("p (w s o) -> p w s o", w=nw, o=D)
                    nh = pool.tile([128, nw * D * h], mybir.dt.bfloat16,
                                   tag=f"h{lvl}_{ri}", name="nh")
                    nc.vector.tensor_tensor(
                        out=nh[:], in0=v[:, :, 0:h, :], in1=v[:, :, h:2 * h, :],
                        op=mybir.AluOpType.add)
                    cur = nh
                    r = h
                    lvl += 1
                for t, sz, si in resid:
                    vres = t[:].rearrange("p (w s o) -> p w s o", w=nw, o=D)
                    vcur = cur[:].rearrange("p (w s o) -> p w s o", w=nw, o=D)
                    nh = pool.tile([128, nw * D], odt, tag=f"h{lvl}_{ri}",
                                   name="nh")
                    nc.vector.tensor_tensor(
                        out=nh[:].rearrange("p (w s o) -> p w s o", w=nw, o=D),
                        in0=vcur[:, :, 0:1, :], in1=vres[:, :, si:si + 1, :],
                        op=mybir.AluOpType.add)
                    cur = nh
                    lvl += 1
                return cur

            relus = []                      # (k0, k1, acc_t) per run
            deferred = []                   # (release_oi, ri, nw, cur, r)
            for oi, ri in enumerate(order):
                while deferred and deferred[0][0] <= oi:
                    _, dri, dnw, dcur, dr = deferred.pop(0)
                    dk = runs[dri]
                    acc = dve_chain(dri, dnw, dcur, dr, [], 1)
                    relus.append((dk[1], dk[2], acc))
                dn, k0, k1, poff = runs[ri]
                nw = k1 - k0
                if ri in gp_l1 and dn % 2 == 0:
                    h = dn // 2
                    v = msg_ts[ri][:].rearrange("p (w s o) -> p w s o",
                                                w=nw, o=D)
                    nh = pool.tile([128, nw * D * h], mybir.dt.bfloat16,
                                   tag=f"h0_{ri}", name="nh")
                    nc.gpsimd.tensor_tensor(
                        out=nh[:], in0=v[:, :, 0:h, :], in1=v[:, :, h:2 * h, :],
                        op=mybir.AluOpType.add)
                    deferred.append((oi + 2, ri, nw, nh, h))
                else:
                    acc = dve_chain(ri, nw, msg_ts[ri], dn, [], 0)
                    relus.append((k0, k1, acc))
            for _, dri, dnw, dcur, dr in deferred:
                dk = runs[dri]
                acc = dve_chain(dri, dnw, dcur, dr, [], 1)
                relus.append((dk[1], dk[2], acc))

            # relu on the scalar engine in window order; store each
            # output third as soon as the relus covering it have run
            relus.sort(key=lambda t: t[0])
            stored = 0
            for k0, k1, acc_t in relus:
                k = k0
                while k < k1:
                    ti = max(i for i in range(3) if thirds[i] <= k)
                    ke = min(k1, thirds[ti + 1])
                    nc.scalar.activation(
                        out_ts[ti][:, (k - thirds[ti]) * D:(ke - thirds[ti]) * D],
                        acc_t[:, (k - k0) * D:(ke - k0) * D],
                        mybir.ActivationFunctionType.Relu)
                    k = ke
                while stored < 3 and k1 >= thirds[stored + 1]:
                    i = stored
                    nc.scalar.dma_start(
                        out=OUTP[:, thirds[i] * D:thirds[i + 1] * D],
                        in_=out_ts[i][:])
                    stored += 1
    nc.compile()
    return nc


def _prep_a(u_s, widx_s, x_bf, qof, tof, qcolofs, TCP):
    """Pack one core's gathered x rows into the A-side matmul operand.

    Returns (XU [128, TCP] bf16, col(edge), j(edge)) where edge order is
    the stable widx sort of this core's edges.
    """
    ordA = np.argsort(widx_s, kind="stable")
    wA = widx_s[ordA]
    n = u_s.shape[0]
    cnts = np.bincount(wA, minlength=NW)
    starts = np.zeros(NW + 1, np.int64)
    np.cumsum(cnts, out=starts[1:])
    rank = np.arange(n) - starts[wA]
    col = qcolofs[qof[wA]] + rank // 2
    j = 2 * tof[wA] + rank % 2

    xu3 = np.zeros((TCP * 8, D), BF16)
    xu3[col * 8 + j] = x_bf[u_s[ordA]]
    # [TCP, 8, 16] -> [8, 16, TCP] -> [128, TCP], row = 16j+i
    XU = np.ascontiguousarray(
        xu3.reshape(TCP, 8, D).transpose(1, 2, 0).reshape(128, TCP)
    )
    col_of_edge = np.empty(n, np.int64)
    col_of_edge[ordA] = col
    j_of_edge = np.empty(n, np.int64)
    j_of_edge[ordA] = j
    return XU, col_of_edge, j_of_edge


def prep_all(x, W, u, v, widx):
    """Host-side layout shared by kernel() and benchmarks."""
    x = np.asarray(x, np.float32)
    W = np.asarray(W, np.float32)
    u = np.asarray(u).astype(np.int64)
    v = np.asarray(v).astype(np.int64)
    widx = np.asarray(widx).astype(np.int64)
    x_bf = x.astype(BF16)

    # ---- shard by destination range -----------------------------------
    shard = v // VSH
    sel = [shard == m for m in range(N_CORES)]
    u_s = [u[s] for s in sel]
    v_s = [v[s] - m * VSH for m, s in enumerate(sel)]
    w_s = [widx[s] for s in sel]

    # ---- A-side quad structure (common across cores) ------------------
    cnts = np.stack([np.bincount(ws, minlength=NW) for ws in w_s])
    nc2 = np.maximum((cnts.max(axis=0) + 1) // 2, 1)   # cols per group
    order = np.argsort(nc2, kind="stable")
    qg = order.reshape(NQ, 4)                          # quad -> 4 groups
    qof = np.empty(NW, np.int64)
    tof = np.empty(NW, np.int64)
    for Q in range(NQ):
        for t in range(4):
            qof[qg[Q, t]] = Q
            tof[qg[Q, t]] = t
    qcols = nc2[qg].max(axis=1)
    qcolofs = np.zeros(NQ + 1, np.int64)
    np.cumsum(qcols, out=qcolofs[1:])
    TC = int(qcolofs[-1])
    TCP = ((TC + 127) // 128) * 128           # ragged last chunk, 128-aligned

    pieces = []
    for Q in range(NQ):
        qs = int(qcolofs[Q])
        qe = qs + int(qcols[Q])
        c = qs
        while c < qe:
            lim = min(qe, (c // CHUNK + 1) * CHUNK, qs + ((c - qs) // PSB + 1) * PSB)
            pieces.append((Q, (c - qs) // PSB, c, lim))
            c = lim
    pieces.sort(key=lambda pc: pc[2])

    # quad weight bank: W8[16j+i, 64o + Q] = W[qg[Q, j//2], o, i]
    A = W[qg]                                          # [NQ, 4, D, D] (Q,t,o,i)
    arr = A.transpose(1, 3, 2, 0)                      # [t, i, o, Q]
    W8 = np.ascontiguousarray(
        np.repeat(arr, 2, axis=0).reshape(128, D * NQ)).astype(BF16)

    # ---- B-side degree-sorted window structure ------------------------
    degs = np.stack([np.bincount(vs, minlength=VSH) for vs in v_s])
    perms = [np.argsort(-degs[m], kind="stable") for m in range(N_CORES)]
    sdeg = np.stack([degs[m][perms[m]] for m in range(N_CORES)])
    DN = sdeg[:, ::128].max(axis=0).astype(np.int64)
    DN = (np.maximum(DN, 1) + 1) // 2 * 2      # mult of 2 for add-halving
    poff = np.zeros(NWIN + 1, np.int64)
    np.cumsum(DN * D, out=poff[1:])
    PT = int(poff[-1])

    runs = []
    k = 0
    while k < NWIN:
        k2 = k
        while k2 < NWIN and DN[k2] == DN[k]:
            k2 += 1
        dn = int(DN[k])
        max_nw = max(1, B_MAX_FREE // (D * dn))
        while k < k2:
            k1 = min(k2, k + max_nw)
            runs.append((dn, k, k1, int(poff[k])))
            k = k1

    prepsA = [_prep_a(u_s[m], w_s[m], x_bf, qof, tof, qcolofs, TCP)
              for m in range(N_CORES)]
    return dict(x_bf=x_bf, u_s=u_s, v_s=v_s, w_s=w_s, W8=W8, TCP=TCP,
                pieces=pieces, degs=degs, perms=perms, DN=DN, poff=poff,
                PT=PT, runs=runs, prepsA=prepsA)


def make_in_maps_b(P, resA):
    """Permute launch A's messages into the B-side window layout."""
    in_maps_b = []
    for m in range(N_CORES):
        msgsA = resA.results[m]["MSG"]                # [128, TCP] bf16
        _, col, j = P["prepsA"][m]
        vecs = msgsA[(j * D)[:, None] + np.arange(D)[None, :], col[:, None]]

        vs = P["v_s"][m]
        ordB = np.argsort(vs, kind="stable")
        vB = vs[ordB]
        deg = P["degs"][m]
        startsB = np.zeros(VSH + 1, np.int64)
        np.cumsum(deg, out=startsB[1:])
        s_of = np.arange(vB.shape[0]) - startsB[vB]   # slot within node
        rank_of_node = np.empty(VSH, np.int64)
        rank_of_node[P["perms"][m]] = np.arange(VSH)
        r = rank_of_node[vB]
        kw = r // 128
        p = r % 128
        base = P["poff"][kw] + s_of * D       # o innermost: [p, w, s, o]
        flat = np.zeros((128, P["PT"]), BF16)
        flat[p[:, None], base[:, None] + np.arange(D)[None, :]] = vecs[ordB]
        in_maps_b.append({"MSGB": flat})
    return in_maps_b


def kernel(x, W, u, v, widx):
    P = prep_all(x, W, u, v, widx)

    # ---- launch A: per-edge transform ---------------------------------
    ncA = _build_kernel_a(P["TCP"], P["pieces"])
    in_maps_a = [{"XU": p[0], "W8": P["W8"]} for p in P["prepsA"]]
    LAST_EXEC_NS.clear()
    resA = run_bass_kernel_spmd(ncA, in_maps_a, list(range(N_CORES)), trace=TRACE)
    if TRACE:
        LAST_EXEC_NS.append(resA.exec_time_ns)

    # ---- host: permute messages widx-order -> v-order -----------------
    in_maps_b = make_in_maps_b(P, resA)

    # ---- launch B: segment-sum + ReLU ---------------------------------
    ncB = _build_kernel_b(P["PT"], P["runs"])
    resB = run_bass_kernel_spmd(ncB, in_maps_b, list(range(N_CORES)), trace=TRACE)
    if TRACE:
        LAST_EXEC_NS.append(resB.exec_time_ns)

    out = np.empty((N_NODES, D), np.float32)
    for m in range(N_CORES):
        outP = resB.results[m]["OUTP"]                # [128, NWIN*D]
        byrank = outP.reshape(128, NWIN, D).transpose(1, 0, 2).reshape(NWIN * 128, D)
        out[m * VSH + P["perms"][m]] = byrank[:VSH].astype(np.float32)
    return out



# revision 24
# speedup vs baseline: 1.0303x; 1.0303x over previous
"""GNN message-passing (R-GCN style) kernel for 8 Trainium2 NeuronCores.

Reference computation:
    msgs = einsum("eoi,ei->eo", W[widx], x[u])      # per-edge transform
    out  = relu(segment_sum(msgs, v, N))            # scatter-add + relu

Distribution strategy: edges are sharded by destination-node range
(12500 nodes per core), so each core owns a disjoint slice of the output
and no inter-core collective is needed.  W and x are replicated.

Device-side work (all FLOPs):
  Launch A: per-edge weight transform.  Weight groups are packed four to
    a matmul ("quads", paired by size so padding stays small): the
    [128,128] block-diagonal lhsT holds each quad member's 16x16 weight
    on two of the eight j-slots, and each rhs column carries 8 edges
    (2 per member group).  This quarters the stationary-weight traffic
    through the PE (the dominant tensor cost) versus one group per
    matmul, and shrinks the SBUF operand to 2.1MB so the zero-fill is
    cheap.  The operand layout bd[16j+i, 1024j + 64o + Q] makes every
    lhsT a single-stride access pattern AND every stripe build a fully
    contiguous [16, 1024] DMA from the 0.26MB host bank W8.  Quad column
    ranges are sized to the actual per-group edge counts (maxed across
    cores so one SPMD program serves all 8 cores).  Each quad owns one
    [128, 512] PSUM tile drained right after its matmul pieces by copies
    alternating between the vector and scalar engines.
  Launch B: segment-sum + ReLU.  Destination nodes are bucketed into
    128-node windows by descending degree, so each window is padded only
    to its own max degree DN_k (rounded to a multiple of 4); two
    pairwise bf16 add levels (DVE 2x mode) halve the slots twice, then a
    short X-reduce finishes each window batch.  The input is stored
    partition-major so every window-run load is one contiguous 2D DMA.
    ReLU on the scalar engine, one contiguous output store at the end.

The host does data layout only: sharding, sorting/padding into the
static structures, gathering x rows into the packed matmul operand, and
permuting the 16-float messages from widx-order to v-order between the
two launches.  (Device-side per-edge random access is not available:
the loadable GPSIMD ucode libraries are absent and indirect DMA has
32B/row descriptor granularity, far too slow for 200K rows/core.)
"""

import sys

sys.path.insert(0, "/opt/trn_rl_repo")

import numpy as np
import ml_dtypes

try:
    # bass_utils imports antenv.axon_hooks when tracing is requested via
    # env; some images lack that module — register a graceful stub so a
    # BASS_TRACE=1 environment degrades to "no trace" instead of crashing.
    import antenv.axon_hooks  # noqa: F401
except ImportError:
    import types

    import antenv

    _hooks = types.ModuleType("antenv.axon_hooks")
    _hooks._hook = None
    _hooks.set_axon_ntff_profile_hook = lambda h: setattr(_hooks, "_hook", h)
    _hooks.get_axon_ntff_profile_hook = lambda: _hooks._hook
    sys.modules["antenv.axon_hooks"] = _hooks
    antenv.axon_hooks = _hooks

import concourse.bacc as bacc
import concourse.mybir as mybir
import concourse.tile as tile
from concourse.bass_utils import run_bass_kernel_spmd

BF16 = ml_dtypes.bfloat16

# set by test harnesses: when True, launches run with trace=True and
# per-launch exec times land in LAST_EXEC_NS
TRACE = False
LAST_EXEC_NS = []

N_NODES = 100000
D = 16
NW = 256
NQ = NW // 4                      # weight quads (4 groups per matmul)
N_CORES = 8
VSH = N_NODES // N_CORES          # 12500 destination nodes per core

CHUNK = 4096                      # A-side columns per DMA chunk (last ragged)
PSB = 512                         # A-side columns per PSUM tile
NWIN = (VSH + 127) // 128         # 98 destination 128-node windows per core
B_MAX_FREE = 6144                 # B-side max elems/partition per sbuf tile
B_OUT_BF16 = True                 # B-side reduce/relu/output in bf16 (DVE 2x)


def _build_kernel_a(TCP, pieces):
    """pieces = list of (Q, reg, c0, c1) ascending in c0: matmul piece of
    quad Q covering absolute cols [c0, c1), accumulated in PSUM tile
    (Q, reg) and drained immediately after."""
    nc = bacc.Bacc(None, target_bir_lowering=False, debug=False)
    XU = nc.dram_tensor("XU", [128, TCP], mybir.dt.bfloat16, kind="ExternalInput")
    W8 = nc.dram_tensor("W8", [128, D * NQ], mybir.dt.bfloat16, kind="ExternalInput")
    MSG = nc.dram_tensor("MSG", [128, TCP], mybir.dt.bfloat16, kind="ExternalOutput")

    by_chunk = {}
    for pc in pieces:
        by_chunk.setdefault(pc[2] // CHUNK, []).append(pc)
    nchunks = (TCP + CHUNK - 1) // CHUNK      # last chunk may be ragged

    with tile.TileContext(nc) as tc:
        with (
            tc.tile_pool(name="bd", bufs=1) as bdp,
            tc.tile_pool(name="xu", bufs=1) as xu_pool,
            tc.tile_pool(name="sbuf", bufs=1) as pool,
            tc.tile_pool(name="psum", bufs=2, space="PSUM") as psum_pool,
        ):
            # ---- striped quad weight operand in SBUF -------------------
            # bd[16j+i, 1024j + 64o + Q] = W[quad Q member j//2][o, i];
            # stripe j is the contiguous block [16j:16j+16, 1024j:1024j+1024]
            bd = bdp.tile([128, 8 * D * NQ], mybir.dt.bfloat16, tag="bd")
            # 3-way memset split so the zero-fill clears in ~1us per engine
            bdu = bd[:].bitcast(mybir.dt.uint32)      # [128, 4096]
            nc.vector.memset(bdu[:, 0:1376], 0)
            nc.gpsimd.memset(bdu[:, 1376:2752], 0)
            nc.scalar.mul(bdu[:, 2752:4096], bdu[:, 2752:4096], 0.0)
            bdv = bd[:].rearrange("p (m q) -> p m q", q=NQ)

            # all XU chunk loads issued up front on the sync queue so the
            # DMA rings stream back-to-back from the start of the launch
            xu_tiles = []
            for ch in range(nchunks):
                w = min(CHUNK, TCP - ch * CHUNK)
                xu_t = xu_pool.tile([128, w], mybir.dt.bfloat16, tag=f"xu{ch}")
                nc.sync.dma_start(out=xu_t[:], in_=XU[:, ch * CHUNK:ch * CHUNK + w])
                xu_tiles.append(xu_t)
            # stripe builds spread over the scalar/gpsimd queues (their
            # sequencers are otherwise idle; sync is busy with XU)
            stripe_q = [nc.scalar, nc.gpsimd]
            for j in range(8):
                stripe_q[j % 2].dma_start(
                    out=bd[16 * j:16 * (j + 1), 1024 * j:1024 * (j + 1)],
                    in_=W8[16 * j:16 * (j + 1), :],
                )
            ncopy = 0
            nstore = 0
            ps_tiles = {}
            for ch in range(nchunks):
                base = ch * CHUNK
                w = min(CHUNK, TCP - base)
                xu_t = xu_tiles[ch]
                out_t = pool.tile([128, w], mybir.dt.bfloat16, tag=f"out{ch}")
                for Q, reg, c0, c1 in by_chunk.get(ch, []):
                    key = (Q, reg)
                    if key not in ps_tiles:
                        ps_new = psum_pool.tile([128, PSB], mybir.dt.float32,
                                                tag=f"ps{len(ps_tiles) % 4}")
                        ps_tiles[key] = (ps_new, c0)  # c0 = tile col origin
                    ps, orig = ps_tiles[key]
                    nc.tensor.matmul(
                        out=ps[:, c0 - orig:c1 - orig],
                        lhsT=bdv[:, :, Q:Q + 1],
                        rhs=xu_t[:, c0 - base:c1 - base],
                        start=True,
                        stop=True,
                    )
                    if ncopy % 2 == 0:
                        nc.vector.tensor_copy(
                            out_t[:, c0 - base:c1 - base], ps[:, c0 - orig:c1 - orig])
                    else:
                        nc.scalar.copy(
                            out=out_t[:, c0 - base:c1 - base],
                            in_=ps[:, c0 - orig:c1 - orig])
                    ncopy += 1
                # half-chunk stores alternating between the gpsimd (SWDGE)
                # and sync (HWDGE) queues
                for h0, h1 in ((0, w // 2), (w // 2, w)):
                    q = nc.gpsimd if nstore % 2 == 0 else nc.sync
                    q.dma_start(out=MSG[:, base + h0:base + h1],
                                in_=out_t[:, h0:h1])
                    nstore += 1
    nc.compile()
    return nc


def _build_kernel_b(PT, runs):
    """runs = list of (dn, k0, k1, poff) equal-DN window runs (chunked).
    MSGB is partition-major: MSGB[p, poff_k + o*dn_k + s] = slot s of
    component o of the node at rank k*128+p, so every window-run DMA is a
    fully contiguous 2D slice."""
    nc = bacc.Bacc(None, target_bir_lowering=False, debug=False)
    odt = mybir.dt.bfloat16 if B_OUT_BF16 else mybir.dt.float32
    MSGB = nc.dram_tensor("MSGB", [128, PT], mybir.dt.bfloat16, kind="ExternalInput")
    # partition-major output: OUTP[p, k*D+o] = out of node at rank k*128+p
    OUTP = nc.dram_tensor("OUTP", [128, NWIN * D], odt, kind="ExternalOutput")

    nruns = len(runs)
    # output split in thirds (separate tiles) so each third's store can
    # trigger as soon as the relus covering it complete
    thirds = [0, (NWIN // 3), (2 * NWIN) // 3, NWIN]

    with nc.allow_low_precision("node sums of <=44 bf16 terms fit error budget"), \
            tile.TileContext(nc) as tc:
        with (
            tc.tile_pool(name="out", bufs=1) as outp,
            tc.tile_pool(name="msg", bufs=1) as msgp,
            tc.tile_pool(name="sbuf", bufs=1) as pool,
        ):
            out_ts = []
            for i in range(3):
                out_t = outp.tile([128, (thirds[i + 1] - thirds[i]) * D], odt,
                                  tag=f"out{i}")
                out_ts.append(out_t)

            # loads issued in a size pyramid (small, ..., big, ..., small)
            # on the sync queue (HWDGE): the first tile lands early so the
            # DVE starts ~9us, cumulative loads stay ahead of the adds,
            # and the last tile is small so the compute tail is short
            asc = sorted(range(nruns),
                         key=lambda i: (runs[i][2] - runs[i][1]) * runs[i][0])
            order = asc[0::2] + asc[1::2][::-1]
            msg_ts = {}
            for ri in order:
                dn, k0, k1, poff = runs[ri]
                nw = k1 - k0
                msg_t = msgp.tile([128, nw * D * dn], mybir.dt.bfloat16,
                                  tag=f"msg{ri}")
                nc.sync.dma_start(out=msg_t[:], in_=MSGB[:, poff:poff + nw * D * dn])
                msg_ts[ri] = msg_t

            # DVE p-state warmup: a few tiny adds so the engine clock has
            # ramped before the first real level-1 lands (a cold first
            # instruction was observed running ~5x slow)
            wrm = pool.tile([128, 64], mybir.dt.bfloat16, tag="wrm")
            nc.vector.memset(wrm[:], 0)
            for _ in range(6):
                nc.vector.tensor_tensor(out=wrm[:, :32], in0=wrm[:, :32],
                                        in1=wrm[:, 32:], op=mybir.AluOpType.add)

            # add tree per run, o innermost ([p, w, s, o]): every split
            # keeps a contiguous 16-wide o run, so ALL levels (including
            # odd-remainder merges) hit the DVE 2x mode.  Odd slot counts
            # park the unpaired plane on a residual list merged at the
            # end, also at 2x.
            # level 1 of the two largest runs goes to the gpsimd engine
            # (mid-pyramid, parallel to the DVE stream); their remaining
            # DVE levels are DEFERRED two runs so the in-order DVE queue
            # never head-of-line blocks on the slow gpsimd add
            gp_l1 = set()

            def dve_chain(ri, nw, cur, r, resid, lvl):
                while r > 1:
                    h = r // 2
                    if r % 2 == 1:
                        resid.append((cur, r, r - 1))
                    v = cur[:].rearrange("p (w s o) -> p w s o", w=nw, o=D)
                    nh = pool.tile([128, nw * D * h], mybir.dt.bfloat16,
                                   tag=f"h{lvl}_{ri}", name="nh")
                    nc.vector.tensor_tensor(
                        out=nh[:], in0=v[:, :, 0:h, :], in1=v[:, :, h:2 * h, :],
                        op=mybir.AluOpType.add)
                    cur = nh
                    r = h
                    lvl += 1
                for t, sz, si in resid:
                    vres = t[:].rearrange("p (w s o) -> p w s o", w=nw, o=D)
                    vcur = cur[:].rearrange("p (w s o) -> p w s o", w=nw, o=D)
                    nh = pool.tile([128, nw * D], odt, tag=f"h{lvl}_{ri}",
                                   name="nh")
                    nc.vector.tensor_tensor(
                        out=nh[:].rearrange("p (w s o) -> p w s o", w=nw, o=D),
                        in0=vcur[:, :, 0:1, :], in1=vres[:, :, si:si + 1, :],
                        op=mybir.AluOpType.add)
                    cur = nh
                    lvl += 1
                return cur

            relus = []                      # (k0, k1, acc_t) per run
            deferred = []                   # (release_oi, ri, nw, cur, r)
            for oi, ri in enumerate(order):
                while deferred and deferred[0][0] <= oi:
                    _, dri, dnw, dcur, dr = deferred.pop(0)
                    dk = runs[dri]
                    acc = dve_chain(dri, dnw, dcur, dr, [], 1)
                    relus.append((dk[1], dk[2], acc))
                dn, k0, k1, poff = runs[ri]
                nw = k1 - k0
                if ri in gp_l1 and dn % 2 == 0:
                    h = dn // 2
                    v = msg_ts[ri][:].rearrange("p (w s o) -> p w s o",
                                                w=nw, o=D)
                    nh = pool.tile([128, nw * D * h], mybir.dt.bfloat16,
                                   tag=f"h0_{ri}", name="nh")
                    nc.gpsimd.tensor_tensor(
                        out=nh[:], in0=v[:, :, 0:h, :], in1=v[:, :, h:2 * h, :],
                        op=mybir.AluOpType.add)
                    deferred.append((oi + 2, ri, nw, nh, h))
                else:
                    acc = dve_chain(ri, nw, msg_ts[ri], dn, [], 0)
                    relus.append((k0, k1, acc))
            for _, dri, dnw, dcur, dr in deferred:
                dk = runs[dri]
                acc = dve_chain(dri, dnw, dcur, dr, [], 1)
                relus.append((dk[1], dk[2], acc))

            # relu on the scalar engine in window order; store each
            # output third as soon as the relus covering it have run
            relus.sort(key=lambda t: t[0])
            stored = 0
            for k0, k1, acc_t in relus:
                k = k0
                while k < k1:
                    ti = max(i for i in range(3) if thirds[i] <= k)
                    ke = min(k1, thirds[ti + 1])
                    nc.scalar.activation(
                        out_ts[ti][:, (k - thirds[ti]) * D:(ke - thirds[ti]) * D],
                        acc_t[:, (k - k0) * D:(ke - k0) * D],
                        mybir.ActivationFunctionType.Relu)
                    k = ke
                while stored < 3 and k1 >= thirds[stored + 1]:
                    i = stored
                    nc.scalar.dma_start(
                        out=OUTP[:, thirds[i] * D:thirds[i + 1] * D],
                        in_=out_ts[i][:])
                    stored += 1
    nc.compile()
    return nc


def _prep_a(u_s, widx_s, x_bf, qof, tof, qcolofs, TCP):
    """Pack one core's gathered x rows into the A-side matmul operand.

    Returns (XU [128, TCP] bf16, col(edge), j(edge)) where edge order is
    the stable widx sort of this core's edges.
    """
    ordA = np.argsort(widx_s, kind="stable")
    wA = widx_s[ordA]
    n = u_s.shape[0]
    cnts = np.bincount(wA, minlength=NW)
    starts = np.zeros(NW + 1, np.int64)
    np.cumsum(cnts, out=starts[1:])
    rank = np.arange(n) - starts[wA]
    col = qcolofs[qof[wA]] + rank // 2
    j = 2 * tof[wA] + rank % 2

    xu3 = np.zeros((TCP * 8, D), BF16)
    xu3[col * 8 + j] = x_bf[u_s[ordA]]
    # [TCP, 8, 16] -> [8, 16, TCP] -> [128, TCP], row = 16j+i
    XU = np.ascontiguousarray(
        xu3.reshape(TCP, 8, D).transpose(1, 2, 0).reshape(128, TCP)
    )
    col_of_edge = np.empty(n, np.int64)
    col_of_edge[ordA] = col
    j_of_edge = np.empty(n, np.int64)
    j_of_edge[ordA] = j
    return XU, col_of_edge, j_of_edge


def prep_all(x, W, u, v, widx):
    """Host-side layout shared by kernel() and benchmarks."""
    x = np.asarray(x, np.float32)
    W = np.asarray(W, np.float32)
    u = np.asarray(u).astype(np.int64)
    v = np.asarray(v).astype(np.int64)
    widx = np.asarray(widx).astype(np.int64)
    x_bf = x.astype(BF16)

    # ---- shard by destination range -----------------------------------
    shard = v // VSH
    sel = [shard == m for m in range(N_CORES)]
    u_s = [u[s] for s in sel]
    v_s = [v[s] - m * VSH for m, s in enumerate(sel)]
    w_s = [widx[s] for s in sel]

    # ---- A-side quad structure (common across cores) ------------------
    cnts = np.stack([np.bincount(ws, minlength=NW) for ws in w_s])
    nc2 = np.maximum((cnts.max(axis=0) + 1) // 2, 1)   # cols per group
    order = np.argsort(nc2, kind="stable")
    qg = order.reshape(NQ, 4)                          # quad -> 4 groups
    qof = np.empty(NW, np.int64)
    tof = np.empty(NW, np.int64)
    for Q in range(NQ):
        for t in range(4):
            qof[qg[Q, t]] = Q
            tof[qg[Q, t]] = t
    qcols = nc2[qg].max(axis=1)
    qcolofs = np.zeros(NQ + 1, np.int64)
    np.cumsum(qcols, out=qcolofs[1:])
    TC = int(qcolofs[-1])
    TCP = ((TC + 127) // 128) * 128           # ragged last chunk, 128-aligned

    pieces = []
    for Q in range(NQ):
        qs = int(qcolofs[Q])
        qe = qs + int(qcols[Q])
        c = qs
        while c < qe:
            lim = min(qe, (c // CHUNK + 1) * CHUNK, qs + ((c - qs) // PSB + 1) * PSB)
            pieces.append((Q, (c - qs) // PSB, c, lim))
            c = lim
    pieces.sort(key=lambda pc: pc[2])

    # quad weight bank: W8[16j+i, 64o + Q] = W[qg[Q, j//2], o, i]
    A = W[qg]                                          # [NQ, 4, D, D] (Q,t,o,i)
    arr = A.transpose(1, 3, 2, 0)                      # [t, i, o, Q]
    W8 = np.ascontiguousarray(
        np.repeat(arr, 2, axis=0).reshape(128, D * NQ)).astype(BF16)

    # ---- B-side degree-sorted window structure ------------------------
    degs = np.stack([np.bincount(vs, minlength=VSH) for vs in v_s])
    perms = [np.argsort(-degs[m], kind="stable") for m in range(N_CORES)]
    sdeg = np.stack([degs[m][perms[m]] for m in range(N_CORES)])
    DN = sdeg[:, ::128].max(axis=0).astype(np.int64)
    DN = (np.maximum(DN, 1) + 1) // 2 * 2      # mult of 2 for add-halving
    poff = np.zeros(NWIN + 1, np.int64)
    np.cumsum(DN * D, out=poff[1:])
    PT = int(poff[-1])

    runs = []
    k = 0
    while k < NWIN:
        k2 = k
        while k2 < NWIN and DN[k2] == DN[k]:
            k2 += 1
        dn = int(DN[k])
        max_nw = max(1, B_MAX_FREE // (D * dn))
        while k < k2:
            k1 = min(k2, k + max_nw)
            runs.append((dn, k, k1, int(poff[k])))
            k = k1

    prepsA = [_prep_a(u_s[m], w_s[m], x_bf, qof, tof, qcolofs, TCP)
              for m in range(N_CORES)]
    return dict(x_bf=x_bf, u_s=u_s, v_s=v_s, w_s=w_s, W8=W8, TCP=TCP,
                pieces=pieces, degs=degs, perms=perms, DN=DN, poff=poff,
                PT=PT, runs=runs, prepsA=prepsA)


def make_in_maps_b(P, resA):
    """Permute launch A's messages into the B-side window layout."""
    in_maps_b = []
    for m in range(N_CORES):
        msgsA = resA.results[m]["MSG"]                # [128, TCP] bf16
        _, col, j = P["prepsA"][m]
        vecs = msgsA[(j * D)[:, None] + np.arange(D)[None, :], col[:, None]]

        vs = P["v_s"][m]
        ordB = np.argsort(vs, kind="stable")
        vB = vs[ordB]
        deg = P["degs"][m]
        startsB = np.zeros(VSH + 1, np.int64)
        np.cumsum(deg, out=startsB[1:])
        s_of = np.arange(vB.shape[0]) - startsB[vB]   # slot within node
        rank_of_node = np.empty(VSH, np.int64)
        rank_of_node[P["perms"][m]] = np.arange(VSH)
        r = rank_of_node[vB]
        kw = r // 128
        p = r % 128
        base = P["poff"][kw] + s_of * D       # o innermost: [p, w, s, o]
        flat = np.zeros((128, P["PT"]), BF16)
        flat[p[:, None], base[:, None] + np.arange(D)[None, :]] = vecs[ordB]
        in_maps_b.append({"MSGB": flat})
    return in_maps_b


def kernel(x, W, u, v, widx):
    P = prep_all(x, W, u, v, widx)

    # ---- launch A: per-edge transform ---------------------------------
    ncA = _build_kernel_a(P["TCP"], P["pieces"])
    in_maps_a = [{"XU": p[0], "W8": P["W8"]} for p in P["prepsA"]]
    LAST_EXEC_NS.clear()
    resA = run_bass_kernel_spmd(ncA, in_maps_a, list(range(N_CORES)), trace=TRACE)
    if TRACE:
        LAST_EXEC_NS.append(resA.exec_time_ns)

    # ---- host: permute messages widx-order -> v-order -----------------
    in_maps_b = make_in_maps_b(P, resA)

    # ---- launch B: segment-sum + ReLU ---------------------------------
    ncB = _build_kernel_b(P["PT"], P["runs"])
    resB = run_bass_kernel_spmd(ncB, in_maps_b, list(range(N_CORES)), trace=TRACE)
    if TRACE:
        LAST_EXEC_NS.append(resB.exec_time_ns)

    out = np.empty((N_NODES, D), np.float32)
    for m in range(N_CORES):
        outP = resB.results[m]["OUTP"]                # [128, NWIN*D]
        byrank = outP.reshape(128, NWIN, D).transpose(1, 0, 2).reshape(NWIN * 128, D)
        out[m * VSH + P["perms"][m]] = byrank[:VSH].astype(np.float32)
    return out



# revision 27
# speedup vs baseline: 1.0985x; 1.0662x over previous
"""GNN message-passing (R-GCN style) kernel for 8 Trainium2 NeuronCores.

Reference computation:
    msgs = einsum("eoi,ei->eo", W[widx], x[u])      # per-edge transform
    out  = relu(segment_sum(msgs, v, N))            # scatter-add + relu

Distribution strategy: edges are sharded by destination-node range
(12500 nodes per core), so each core owns a disjoint slice of the output
and no inter-core collective is needed.  W and x are replicated.

Device-side work (all FLOPs):
  Launch A: per-edge weight transform.  Weight groups are packed four to
    a matmul ("quads", paired by size so padding stays small): the
    [128,128] block-diagonal lhsT holds each quad member's 16x16 weight
    on two of the eight j-slots, and each rhs column carries 8 edges
    (2 per member group).  This quarters the stationary-weight traffic
    through the PE (the dominant tensor cost) versus one group per
    matmul, and shrinks the SBUF operand to 2.1MB so the zero-fill is
    cheap.  The operand layout bd[16j+i, 1024j + 64o + Q] makes every
    lhsT a single-stride access pattern AND every stripe build a fully
    contiguous [16, 1024] DMA from the 0.26MB host bank W8.  Quad column
    ranges are sized to the actual per-group edge counts (maxed across
    cores so one SPMD program serves all 8 cores).  Each quad owns one
    [128, 512] PSUM tile drained right after its matmul pieces by copies
    alternating between the vector and scalar engines.
  Launch B: segment-sum + ReLU.  Destination nodes are bucketed into
    128-node windows by descending degree, so each window is padded only
    to its own max degree DN_k (rounded to a multiple of 4); two
    pairwise bf16 add levels (DVE 2x mode) halve the slots twice, then a
    short X-reduce finishes each window batch.  The input is stored
    partition-major so every window-run load is one contiguous 2D DMA.
    ReLU on the scalar engine, one contiguous output store at the end.

The host does data layout only: sharding, sorting/padding into the
static structures, gathering x rows into the packed matmul operand, and
permuting the 16-float messages from widx-order to v-order between the
two launches.  (Device-side per-edge random access is not available:
the loadable GPSIMD ucode libraries are absent and indirect DMA has
32B/row descriptor granularity, far too slow for 200K rows/core.)
"""

import sys

sys.path.insert(0, "/opt/trn_rl_repo")

import numpy as np
import ml_dtypes

try:
    # bass_utils imports antenv.axon_hooks when tracing is requested via
    # env; some images lack that module — register a graceful stub so a
    # BASS_TRACE=1 environment degrades to "no trace" instead of crashing.
    import antenv.axon_hooks  # noqa: F401
except ImportError:
    import types

    import antenv

    _hooks = types.ModuleType("antenv.axon_hooks")
    _hooks._hook = None
    _hooks.set_axon_ntff_profile_hook = lambda h: setattr(_hooks, "_hook", h)
    _hooks.get_axon_ntff_profile_hook = lambda: _hooks._hook
    sys.modules["antenv.axon_hooks"] = _hooks
    antenv.axon_hooks = _hooks

import concourse.bacc as bacc
import concourse.mybir as mybir
import concourse.tile as tile
from concourse.bass_utils import run_bass_kernel_spmd

BF16 = ml_dtypes.bfloat16

# set by test harnesses: when True, launches run with trace=True and
# per-launch exec times land in LAST_EXEC_NS
TRACE = False
LAST_EXEC_NS = []

N_NODES = 100000
D = 16
NW = 256
NQ = NW // 4                      # weight quads (4 groups per matmul)
N_CORES = 8
VSH = N_NODES // N_CORES          # 12500 destination nodes per core

CHUNK = 4096                      # A-side columns per DMA chunk (last ragged)
PSB = 512                         # A-side columns per PSUM tile
NWIN = (VSH + 127) // 128         # 98 destination 128-node windows per core
B_MAX_FREE = 6144                 # B-side max elems/partition per sbuf tile
B_OUT_BF16 = True                 # B-side reduce/relu/output in bf16 (DVE 2x)


def _build_kernel_a(TCP, pieces):
    """pieces = list of (Q, reg, c0, c1) ascending in c0: matmul piece of
    quad Q covering absolute cols [c0, c1), accumulated in PSUM tile
    (Q, reg) and drained immediately after."""
    nc = bacc.Bacc(None, target_bir_lowering=False, debug=False)
    XU = nc.dram_tensor("XU", [128, TCP], mybir.dt.bfloat16, kind="ExternalInput")
    W8 = nc.dram_tensor("W8", [128, D * NQ], mybir.dt.bfloat16, kind="ExternalInput")
    MSG = nc.dram_tensor("MSG", [128, TCP], mybir.dt.bfloat16, kind="ExternalOutput")

    by_chunk = {}
    for pc in pieces:
        by_chunk.setdefault(pc[2] // CHUNK, []).append(pc)
    nchunks = (TCP + CHUNK - 1) // CHUNK      # last chunk may be ragged

    with tile.TileContext(nc) as tc:
        with (
            tc.tile_pool(name="bd", bufs=1) as bdp,
            tc.tile_pool(name="xu", bufs=1) as xu_pool,
            tc.tile_pool(name="sbuf", bufs=1) as pool,
            tc.tile_pool(name="psum", bufs=2, space="PSUM") as psum_pool,
        ):
            # ---- striped quad weight operand in SBUF -------------------
            # bd[16j+i, 1024j + 64o + Q] = W[quad Q member j//2][o, i];
            # stripe j is the contiguous block [16j:16j+16, 1024j:1024j+1024]
            bd = bdp.tile([128, 8 * D * NQ], mybir.dt.bfloat16, tag="bd")
            # 3-way memset split so the zero-fill clears in ~1us per engine
            bdu = bd[:].bitcast(mybir.dt.uint32)      # [128, 4096]
            nc.vector.memset(bdu[:, 0:1376], 0)
            nc.gpsimd.memset(bdu[:, 1376:2752], 0)
            nc.scalar.mul(bdu[:, 2752:4096], bdu[:, 2752:4096], 0.0)
            bdv = bd[:].rearrange("p (m q) -> p m q", q=NQ)

            # all XU chunk loads issued up front on the sync queue so the
            # DMA rings stream back-to-back from the start of the launch
            xu_tiles = []
            for ch in range(nchunks):
                w = min(CHUNK, TCP - ch * CHUNK)
                xu_t = xu_pool.tile([128, w], mybir.dt.bfloat16, tag=f"xu{ch}")
                nc.sync.dma_start(out=xu_t[:], in_=XU[:, ch * CHUNK:ch * CHUNK + w])
                xu_tiles.append(xu_t)
            # stripe builds spread over the scalar/gpsimd queues (their
            # sequencers are otherwise idle; sync is busy with XU)
            stripe_q = [nc.scalar, nc.gpsimd]
            for j in range(8):
                stripe_q[j % 2].dma_start(
                    out=bd[16 * j:16 * (j + 1), 1024 * j:1024 * (j + 1)],
                    in_=W8[16 * j:16 * (j + 1), :],
                )
            ncopy = 0
            nstore = 0
            ps_tiles = {}
            for ch in range(nchunks):
                base = ch * CHUNK
                w = min(CHUNK, TCP - base)
                xu_t = xu_tiles[ch]
                out_t = pool.tile([128, w], mybir.dt.bfloat16, tag=f"out{ch}")
                for Q, reg, c0, c1 in by_chunk.get(ch, []):
                    key = (Q, reg)
                    if key not in ps_tiles:
                        ps_new = psum_pool.tile([128, PSB], mybir.dt.float32,
                                                tag=f"ps{len(ps_tiles) % 4}")
                        ps_tiles[key] = (ps_new, c0)  # c0 = tile col origin
                    ps, orig = ps_tiles[key]
                    nc.tensor.matmul(
                        out=ps[:, c0 - orig:c1 - orig],
                        lhsT=bdv[:, :, Q:Q + 1],
                        rhs=xu_t[:, c0 - base:c1 - base],
                        start=True,
                        stop=True,
                    )
                    if ncopy % 2 == 0:
                        nc.vector.tensor_copy(
                            out_t[:, c0 - base:c1 - base], ps[:, c0 - orig:c1 - orig])
                    else:
                        nc.scalar.copy(
                            out=out_t[:, c0 - base:c1 - base],
                            in_=ps[:, c0 - orig:c1 - orig])
                    ncopy += 1
                # half-chunk stores alternating between the gpsimd (SWDGE)
                # and sync (HWDGE) queues
                for h0, h1 in ((0, w // 2), (w // 2, w)):
                    q = nc.gpsimd if nstore % 2 == 0 else nc.sync
                    q.dma_start(out=MSG[:, base + h0:base + h1],
                                in_=out_t[:, h0:h1])
                    nstore += 1
    nc.compile()
    return nc


def _build_kernel_b(PT, runs):
    """runs = list of (dn, k0, k1, poff) equal-DN window runs (chunked).
    MSGB is partition-major: MSGB[p, poff_k + o*dn_k + s] = slot s of
    component o of the node at rank k*128+p, so every window-run DMA is a
    fully contiguous 2D slice."""
    nc = bacc.Bacc(None, target_bir_lowering=False, debug=False)
    odt = mybir.dt.bfloat16 if B_OUT_BF16 else mybir.dt.float32
    MSGB = nc.dram_tensor("MSGB", [128, PT], mybir.dt.bfloat16, kind="ExternalInput")
    # partition-major output: OUTP[p, k*D+o] = out of node at rank k*128+p
    OUTP = nc.dram_tensor("OUTP", [128, NWIN * D], odt, kind="ExternalOutput")

    nruns = len(runs)

    with nc.allow_low_precision("node sums of <=44 bf16 terms fit error budget"), \
            tile.TileContext(nc) as tc:
        with (
            tc.tile_pool(name="out", bufs=1) as outp,
            tc.tile_pool(name="msg", bufs=1) as msgp,
            tc.tile_pool(name="sbuf", bufs=1) as pool,
        ):
            # loads issued in a size pyramid (small, ..., big, ..., small)
            # on the sync queue (HWDGE): the first tile lands early so the
            # DVE starts ~9us, cumulative loads stay ahead of the adds,
            # and the last tile is small so the compute tail is short
            asc = sorted(range(nruns),
                         key=lambda i: (runs[i][2] - runs[i][1]) * runs[i][0])
            order = asc[0::2] + asc[1::2][::-1]
            msg_ts = {}
            for ri in order:
                dn, k0, k1, poff = runs[ri]
                nw = k1 - k0
                msg_t = msgp.tile([128, nw * D * dn], mybir.dt.bfloat16,
                                  tag=f"msg{ri}")
                nc.sync.dma_start(out=msg_t[:], in_=MSGB[:, poff:poff + nw * D * dn])
                msg_ts[ri] = msg_t

            # DVE p-state warmup: a few tiny adds so the engine clock has
            # ramped before the first real level-1 lands (a cold first
            # instruction was observed running ~5x slow)
            wrm = pool.tile([128, 64], mybir.dt.bfloat16, tag="wrm")
            nc.vector.memset(wrm[:], 0)
            for _ in range(6):
                nc.vector.tensor_tensor(out=wrm[:, :32], in0=wrm[:, :32],
                                        in1=wrm[:, 32:], op=mybir.AluOpType.add)

            # add tree per run, o innermost ([p, w, s, o]): every split
            # keeps a contiguous 16-wide o run, so ALL levels (including
            # odd-remainder merges) hit the DVE 2x mode.  Odd slot counts
            # park the unpaired plane on a residual list merged at the
            # end, also at 2x.
            # level 1 of the two largest runs goes to the gpsimd engine
            # (mid-pyramid, parallel to the DVE stream); their remaining
            # DVE levels are DEFERRED two runs so the in-order DVE queue
            # never head-of-line blocks on the slow gpsimd add
            gp_l1 = set()

            def dve_chain(ri, nw, cur, r, resid, lvl):
                while r > 1:
                    h = r // 2
                    if r % 2 == 1:
                        resid.append((cur, r, r - 1))
                    v = cur[:].rearrange("p (w s o) -> p w s o", w=nw, o=D)
                    nh = pool.tile([128, nw * D * h], mybir.dt.bfloat16,
                                   tag=f"h{lvl}_{ri}", name="nh")
                    nc.vector.tensor_tensor(
                        out=nh[:], in0=v[:, :, 0:h, :], in1=v[:, :, h:2 * h, :],
                        op=mybir.AluOpType.add)
                    cur = nh
                    r = h
                    lvl += 1
                for t, sz, si in resid:
                    vres = t[:].rearrange("p (w s o) -> p w s o", w=nw, o=D)
                    vcur = cur[:].rearrange("p (w s o) -> p w s o", w=nw, o=D)
                    nh = pool.tile([128, nw * D], odt, tag=f"h{lvl}_{ri}",
                                   name="nh")
                    nc.vector.tensor_tensor(
                        out=nh[:].rearrange("p (w s o) -> p w s o", w=nw, o=D),
                        in0=vcur[:, :, 0:1, :], in1=vres[:, :, si:si + 1, :],
                        op=mybir.AluOpType.add)
                    cur = nh
                    lvl += 1
                return cur

            relus = []                      # (k0, k1, acc_t) per run
            deferred = []                   # (release_oi, ri, nw, cur, r)
            for oi, ri in enumerate(order):
                while deferred and deferred[0][0] <= oi:
                    _, dri, dnw, dcur, dr = deferred.pop(0)
                    dk = runs[dri]
                    acc = dve_chain(dri, dnw, dcur, dr, [], 1)
                    relus.append((dk[1], dk[2], acc))
                dn, k0, k1, poff = runs[ri]
                nw = k1 - k0
                if ri in gp_l1 and dn % 2 == 0:
                    h = dn // 2
                    v = msg_ts[ri][:].rearrange("p (w s o) -> p w s o",
                                                w=nw, o=D)
                    nh = pool.tile([128, nw * D * h], mybir.dt.bfloat16,
                                   tag=f"h0_{ri}", name="nh")
                    nc.gpsimd.tensor_tensor(
                        out=nh[:], in0=v[:, :, 0:h, :], in1=v[:, :, h:2 * h, :],
                        op=mybir.AluOpType.add)
                    deferred.append((oi + 2, ri, nw, nh, h))
                else:
                    acc = dve_chain(ri, nw, msg_ts[ri], dn, [], 0)
                    relus.append((k0, k1, acc))
            for _, dri, dnw, dcur, dr in deferred:
                dk = runs[dri]
                acc = dve_chain(dri, dnw, dcur, dr, [], 1)
                relus.append((dk[1], dk[2], acc))

            # relu on the scalar engine in acc-completion order (k-sorted
            # would head-of-line block the in-order scalar queue on the
            # last run), then store that window range immediately
            for idx, (k0, k1, acc_t) in enumerate(relus):
                out_t = outp.tile([128, (k1 - k0) * D], odt, tag=f"o{idx}",
                                  name="out_t")
                nc.scalar.activation(out_t[:], acc_t[:],
                                     mybir.ActivationFunctionType.Relu)
                nc.scalar.dma_start(out=OUTP[:, k0 * D:k1 * D], in_=out_t[:])
    nc.compile()
    return nc


def _prep_a(u_s, widx_s, x_bf, qof, tof, qcolofs, TCP):
    """Pack one core's gathered x rows into the A-side matmul operand.

    Returns (XU [128, TCP] bf16, col(edge), j(edge)) where edge order is
    the stable widx sort of this core's edges.
    """
    ordA = np.argsort(widx_s, kind="stable")
    wA = widx_s[ordA]
    n = u_s.shape[0]
    cnts = np.bincount(wA, minlength=NW)
    starts = np.zeros(NW + 1, np.int64)
    np.cumsum(cnts, out=starts[1:])
    rank = np.arange(n) - starts[wA]
    col = qcolofs[qof[wA]] + rank // 2
    j = 2 * tof[wA] + rank % 2

    xu3 = np.zeros((TCP * 8, D), BF16)
    xu3[col * 8 + j] = x_bf[u_s[ordA]]
    # [TCP, 8, 16] -> [8, 16, TCP] -> [128, TCP], row = 16j+i
    XU = np.ascontiguousarray(
        xu3.reshape(TCP, 8, D).transpose(1, 2, 0).reshape(128, TCP)
    )
    col_of_edge = np.empty(n, np.int64)
    col_of_edge[ordA] = col
    j_of_edge = np.empty(n, np.int64)
    j_of_edge[ordA] = j
    return XU, col_of_edge, j_of_edge


def prep_all(x, W, u, v, widx):
    """Host-side layout shared by kernel() and benchmarks."""
    x = np.asarray(x, np.float32)
    W = np.asarray(W, np.float32)
    u = np.asarray(u).astype(np.int64)
    v = np.asarray(v).astype(np.int64)
    widx = np.asarray(widx).astype(np.int64)
    x_bf = x.astype(BF16)

    # ---- shard by destination range -----------------------------------
    shard = v // VSH
    sel = [shard == m for m in range(N_CORES)]
    u_s = [u[s] for s in sel]
    v_s = [v[s] - m * VSH for m, s in enumerate(sel)]
    w_s = [widx[s] for s in sel]

    # ---- A-side quad structure (common across cores) ------------------
    cnts = np.stack([np.bincount(ws, minlength=NW) for ws in w_s])
    nc2 = np.maximum((cnts.max(axis=0) + 1) // 2, 1)   # cols per group
    order = np.argsort(nc2, kind="stable")
    qg = order.reshape(NQ, 4)                          # quad -> 4 groups
    qof = np.empty(NW, np.int64)
    tof = np.empty(NW, np.int64)
    for Q in range(NQ):
        for t in range(4):
            qof[qg[Q, t]] = Q
            tof[qg[Q, t]] = t
    qcols = nc2[qg].max(axis=1)
    qcolofs = np.zeros(NQ + 1, np.int64)
    np.cumsum(qcols, out=qcolofs[1:])
    TC = int(qcolofs[-1])
    TCP = ((TC + 127) // 128) * 128           # ragged last chunk, 128-aligned

    pieces = []
    for Q in range(NQ):
        qs = int(qcolofs[Q])
        qe = qs + int(qcols[Q])
        c = qs
        while c < qe:
            lim = min(qe, (c // CHUNK + 1) * CHUNK, qs + ((c - qs) // PSB + 1) * PSB)
            pieces.append((Q, (c - qs) // PSB, c, lim))
            c = lim
    pieces.sort(key=lambda pc: pc[2])

    # quad weight bank: W8[16j+i, 64o + Q] = W[qg[Q, j//2], o, i]
    A = W[qg]                                          # [NQ, 4, D, D] (Q,t,o,i)
    arr = A.transpose(1, 3, 2, 0)                      # [t, i, o, Q]
    W8 = np.ascontiguousarray(
        np.repeat(arr, 2, axis=0).reshape(128, D * NQ)).astype(BF16)

    # ---- B-side degree-sorted window structure ------------------------
    degs = np.stack([np.bincount(vs, minlength=VSH) for vs in v_s])
    perms = [np.argsort(-degs[m], kind="stable") for m in range(N_CORES)]
    sdeg = np.stack([degs[m][perms[m]] for m in range(N_CORES)])
    DN = sdeg[:, ::128].max(axis=0).astype(np.int64)
    DN = (np.maximum(DN, 1) + 1) // 2 * 2      # mult of 2 for add-halving
    poff = np.zeros(NWIN + 1, np.int64)
    np.cumsum(DN * D, out=poff[1:])
    PT = int(poff[-1])

    runs = []
    k = 0
    while k < NWIN:
        k2 = k
        while k2 < NWIN and DN[k2] == DN[k]:
            k2 += 1
        dn = int(DN[k])
        max_nw = max(1, B_MAX_FREE // (D * dn))
        while k < k2:
            k1 = min(k2, k + max_nw)
            runs.append((dn, k, k1, int(poff[k])))
            k = k1

    prepsA = [_prep_a(u_s[m], w_s[m], x_bf, qof, tof, qcolofs, TCP)
              for m in range(N_CORES)]
    return dict(x_bf=x_bf, u_s=u_s, v_s=v_s, w_s=w_s, W8=W8, TCP=TCP,
                pieces=pieces, degs=degs, perms=perms, DN=DN, poff=poff,
                PT=PT, runs=runs, prepsA=prepsA)


def make_in_maps_b(P, resA):
    """Permute launch A's messages into the B-side window layout."""
    in_maps_b = []
    for m in range(N_CORES):
        msgsA = resA.results[m]["MSG"]                # [128, TCP] bf16
        _, col, j = P["prepsA"][m]
        vecs = msgsA[(j * D)[:, None] + np.arange(D)[None, :], col[:, None]]

        vs = P["v_s"][m]
        ordB = np.argsort(vs, kind="stable")
        vB = vs[ordB]
        deg = P["degs"][m]
        startsB = np.zeros(VSH + 1, np.int64)
        np.cumsum(deg, out=startsB[1:])
        s_of = np.arange(vB.shape[0]) - startsB[vB]   # slot within node
        rank_of_node = np.empty(VSH, np.int64)
        rank_of_node[P["perms"][m]] = np.arange(VSH)
        r = rank_of_node[vB]
        kw = r // 128
        p = r % 128
        base = P["poff"][kw] + s_of * D       # o innermost: [p, w, s, o]
        flat = np.zeros((128, P["PT"]), BF16)
        flat[p[:, None], base[:, None] + np.arange(D)[None, :]] = vecs[ordB]
        in_maps_b.append({"MSGB": flat})
    return in_maps_b


def kernel(x, W, u, v, widx):
    P = prep_all(x, W, u, v, widx)

    # ---- launch A: per-edge transform ---------------------------------
    ncA = _build_kernel_a(P["TCP"], P["pieces"])
    in_maps_a = [{"XU": p[0], "W8": P["W8"]} for p in P["prepsA"]]
    LAST_EXEC_NS.clear()
    resA = run_bass_kernel_spmd(ncA, in_maps_a, list(range(N_CORES)), trace=TRACE)
    if TRACE:
        LAST_EXEC_NS.append(resA.exec_time_ns)

    # ---- host: permute messages widx-order -> v-order -----------------
    in_maps_b = make_in_maps_b(P, resA)

    # ---- launch B: segment-sum + ReLU ---------------------------------
    ncB = _build_kernel_b(P["PT"], P["runs"])
    resB = run_bass_kernel_spmd(ncB, in_maps_b, list(range(N_CORES)), trace=TRACE)
    if TRACE:
        LAST_EXEC_NS.append(resB.exec_time_ns)

    out = np.empty((N_NODES, D), np.float32)
    for m in range(N_CORES):
        outP = resB.results[m]["OUTP"]                # [128, NWIN*D]
        byrank = outP.reshape(128, NWIN, D).transpose(1, 0, 2).reshape(NWIN * 128, D)
        out[m * VSH + P["perms"][m]] = byrank[:VSH].astype(np.float32)
    return out

